# revision 8
# baseline (speedup 1.0000x reference)
"""Trainium2 Bass kernel for the SGS turbulence stress (AdvancedTurbulenceHardCore).

Strategy
--------
Shard the [B=2, C=3, 128, 128, 128] velocity field over 8 NeuronCores as
(b, 32-slice D-slab) domains with a 1-voxel replicate-padded halo taken
directly from the full input on the host (no collectives needed).

Per core, the field is processed in groups of G=4 D-slices with layout
[partition = H(128), free = (d, W)].  Work is spread over all four compute
engines:
  - TensorE: d/dH central differences as a banded-matrix matmul, plus all
    linear plane accumulations (scaled-identity weights accumulating into
    PSUM tau banks).
  - VectorE/GpSimd: the remaining differences and per-voxel products
    (fused via scalar_tensor_tensor where possible).
  - ScalarE: squares/sqrts (one ACT table set) and PSUM->SBUF evacuation.

All physics coefficients enter at runtime via a small [128, 8] coefficient
tensor and a [128, 10*128] weight-matrix tensor, so the NEFF is compiled
exactly once regardless of the scalar parameter values.
"""

import os
import sys
import numpy as np

for _p in ("/opt/trn_rl_repo", "/root/.axon_site/_ro/trn_rl_repo"):
    if os.path.isdir(_p) and _p not in sys.path:
        sys.path.insert(0, _p)

B, C, D, H, W = 2, 3, 128, 128, 128
NCORES = 8
DPC = D * B // NCORES      # 32 d-slices per core
G = 4                      # d-slices per compute group
NG = DPC // G
WP = W + 2                 # W padded with replicate halo
SLAB_D = DPC + 2           # d-slab with halo
EPS = 1e-8

_compiled = None           # (nc, run_fn) cache — kernel is value-independent


def _softplus(x):
    x = float(x)
    return float(np.log1p(np.exp(-abs(x))) + max(x, 0.0))


def _dh_matrix():
    """lhsT [K=h, M=h'] with out[h'] = v[h'+1] - v[h'-1] (replicate edges)."""
    m = np.zeros((H, H), np.float32)
    for hp in range(H):
        m[min(hp + 1, H - 1), hp] += 1.0
        m[max(hp - 1, 0), hp] -= 1.0
    return m


# weight-matrix slots in the "wm" input
W_DH, W_I, W_NI, W_N2I, W_23I, W_N13I, W_13I, W_N23I, W_2I, W_KA4 = range(10)
# coefficient columns in the "cf" input
C_HALF_KA, C_SQ_KA, C_SQ_KA_H, C_MP_S, C_MP_B, C_NP_S, C_NP_B, C_SPARE = range(8)


def _build_wm(K_a):
    wm = np.zeros((128, 10 * 128), np.float32)
    eye = np.eye(128, dtype=np.float32)
    wm[:, W_DH * 128:(W_DH + 1) * 128] = _dh_matrix()
    for slot, coef in ((W_I, 1.0), (W_NI, -1.0), (W_N2I, -2.0),
                       (W_23I, 2.0 / 3.0), (W_N13I, -1.0 / 3.0),
                       (W_13I, 1.0 / 3.0), (W_N23I, -2.0 / 3.0),
                       (W_2I, 2.0), (W_KA4, K_a / 4.0)):
        wm[:, slot * 128:(slot + 1) * 128] = coef * eye
    return wm


def _build_cf(P_d, P_r, K_a, epsp):
    cf = np.zeros((128, 8), np.float32)
    cf[:, C_HALF_KA] = K_a / 2.0
    cf[:, C_SQ_KA] = np.sqrt(K_a)
    cf[:, C_SQ_KA_H] = np.sqrt(K_a) / 2.0
    cf[:, C_MP_S] = P_d * P_d / (2.0 * K_a)
    cf[:, C_MP_B] = P_d * P_d / 4.0 * epsp
    cf[:, C_NP_S] = P_r * P_r / 4.0
    cf[:, C_NP_B] = P_r * P_r / 4.0 * epsp
    return cf


def _trace_body(ctx, tc, v_ap, wm_ap, cf_ap, tau_ap):
    import concourse.mybir as mybir
    from concourse.mybir import AluOpType as alu
    from concourse.mybir import ActivationFunctionType as actf

    nc = tc.nc
    f32 = mybir.dt.float32
    SQUARE, SQRT = actf.Square, actf.Sqrt

    const_pool = ctx.enter_context(tc.tile_pool(name="const", bufs=1))
    vin_pool = ctx.enter_context(tc.tile_pool(name="vin", bufs=2))
    pl = ctx.enter_context(tc.tile_pool(name="pl", bufs=2))
    pl1 = ctx.enter_context(tc.tile_pool(name="pl1", bufs=1))
    act_pool = ctx.enter_context(tc.tile_pool(name="actp", bufs=2))
    out_pool = ctx.enter_context(tc.tile_pool(name="outp", bufs=2))
    ps_gy = ctx.enter_context(tc.tile_pool(name="psgy", bufs=3, space="PSUM"))
    ps_gq = ctx.enter_context(tc.tile_pool(name="psgq", bufs=1, space="PSUM"))
    ps_oq = ctx.enter_context(tc.tile_pool(name="psoq", bufs=1, space="PSUM"))
    ps_tau = ctx.enter_context(tc.tile_pool(name="pstau", bufs=3, space="PSUM"))

    wm_t = const_pool.tile([128, 10 * 128], f32, tag="wm", name="wm_t")
    nc.sync.dma_start(wm_t[:], wm_ap[:, :])
    cf_t = const_pool.tile([128, 8], f32, tag="cf", name="cf_t")
    nc.sync.dma_start(cf_t[:], cf_ap[:, :])

    def wmat(slot):
        return wm_t[:, slot * 128:(slot + 1) * 128]

    def coef(col):
        return cf_t[:, col:col + 1]

    for g in range(NG):
        d0 = g * G
        # ---- load the 6-slice window for all 3 components: [128, C, G+2, WP]
        vt = vin_pool.tile([128, C, G + 2, WP], f32, tag="vt", name="vt")
        for c in range(C):
            nc.sync.dma_start(
                vt[:, c],
                v_ap[c, d0:d0 + G + 2, :, :].rearrange("d h w -> h d w"))

        def vx_hi(c):
            return vt[:, c, 1:1 + G, 2:2 + W]

        def vx_lo(c):
            return vt[:, c, 1:1 + G, 0:W]

        def vz_hi(c):
            return vt[:, c, 2:2 + G, 1:1 + W]

        def vz_lo(c):
            return vt[:, c, 0:G, 1:1 + W]

        def vmid(c):
            return vt[:, c, 1:1 + G, 1:1 + W]

        def plane(tag):
            return pl.tile([128, G, W], f32, tag=tag, name=f"{tag}_t")

        def plane1(tag):
            # single-buffered: produced and consumed within one group
            return pl1.tile([128, G, W], f32, tag=tag, name=f"{tag}_t")

        # ---- raw differences -------------------------------------------
        a0 = plane("a0")
        nc.gpsimd.tensor_tensor(a0[:], vx_hi(0), vx_lo(0), alu.subtract)
        a2 = plane("a2")
        nc.gpsimd.tensor_tensor(a2[:], vz_hi(2), vz_lo(2), alu.subtract)
        d0z = plane("d0z")
        nc.gpsimd.tensor_tensor(d0z[:], vz_hi(0), vz_lo(0), alu.subtract)
        d2x = plane("d2x")
        nc.gpsimd.tensor_tensor(d2x[:], vx_hi(2), vx_lo(2), alu.subtract)
        d1x = plane("d1x")
        nc.vector.tensor_tensor(d1x[:], vx_hi(1), vx_lo(1), alu.subtract)
        d1z = plane("d1z")
        nc.vector.tensor_tensor(d1z[:], vz_hi(1), vz_lo(1), alu.subtract)

        gy = []
        for c in range(3):
            t = ps_gy.tile([128, G, W], f32, tag="gy", name="gy_t")
            nc.tensor.matmul(t[:], wmat(W_DH), vmid(c), start=True, stop=True)
            gy.append(t)
        d0y, d1y_ps, d2y = gy

        # a1 out of PSUM early to free the bank and keep GpSimd eligible
        a1 = plane("a1")
        nc.scalar.copy(a1[:], d1y_ps[:])

        # ---- symmetric / antisymmetric combinations --------------------
        p1 = plane("p1")
        nc.vector.tensor_tensor(p1[:], d0y[:], d1x[:], alu.add)
        q1 = plane("q1")
        nc.vector.tensor_tensor(q1[:], d0y[:], d1x[:], alu.subtract)
        p3 = plane("p3")
        nc.vector.tensor_tensor(p3[:], d2y[:], d1z[:], alu.add)
        q3 = plane("q3")
        nc.vector.tensor_tensor(q3[:], d1z[:], d2y[:], alu.subtract)
        p2 = plane("p2")
        nc.gpsimd.tensor_tensor(p2[:], d0z[:], d2x[:], alu.add)
        q2 = plane("q2")
        nc.gpsimd.tensor_tensor(q2[:], d0z[:], d2x[:], alu.subtract)

        avars = (a0, a1, a2)
        pvars = (p1, p2, p3)
        qvars = (q1, q2, q3)

        # ---- squares on ACT (coefficients folded into the scale) -------
        Asq, Psq, Bsq = [], [], []
        for i in range(3):
            t = act_pool.tile([128, G, W], f32, tag=f"asq{i}", name=f"asq{i}_t")
            nc.scalar.activation(t[:], avars[i][:], SQUARE, scale=coef(C_SQ_KA))
            Asq.append(t)
        for k in range(3):
            t = act_pool.tile([128, G, W], f32, tag=f"psq{k}", name=f"psq{k}_t")
            nc.scalar.activation(t[:], pvars[k][:], SQUARE, scale=coef(C_SQ_KA_H))
            Psq.append(t)
        for k in range(3):
            t = act_pool.tile([128, G, W], f32, tag=f"bsq{k}", name=f"bsq{k}_t")
            nc.scalar.activation(t[:], qvars[k][:], SQUARE)
            Bsq.append(t)

        # ---- G = K_a*Q and OsqR via PE accumulation --------------------
        gq = ps_gq.tile([128, G, W], f32, tag="gq", name="gq")
        for j, (wslot, t) in enumerate(
            [(W_I, Asq[0]), (W_I, Asq[1]), (W_I, Asq[2]),
             (W_2I, Psq[0]), (W_2I, Psq[1]), (W_2I, Psq[2])]
        ):
            nc.tensor.matmul(gq[:], wmat(wslot), t[:], start=(j == 0), stop=(j == 5))
        mp = plane("mp")
        nc.scalar.activation(mp[:], gq[:], SQRT, scale=coef(C_MP_S),
                             bias=coef(C_MP_B))

        oq = ps_oq.tile([128, G, W], f32, tag="oq", name="oq")
        for j in range(3):
            nc.tensor.matmul(oq[:], wmat(W_I), Bsq[j][:], start=(j == 0),
                             stop=(j == 2))
        np_ = plane("np")
        nc.scalar.activation(np_[:], oq[:], SQRT, scale=coef(C_NP_S),
                             bias=coef(C_NP_B))

        # ---- per-voxel products ---------------------------------------
        sig1 = plane1("sig1")
        nc.vector.tensor_tensor(sig1[:], a0[:], a1[:], alu.add)
        sig2 = plane1("sig2")
        nc.gpsimd.tensor_tensor(sig2[:], a0[:], a2[:], alu.add)
        sig3 = plane1("sig3")
        nc.vector.tensor_tensor(sig3[:], a1[:], a2[:], alu.add)
        sigs = (sig1, sig2, sig3)

        Fs, zs, ws, x1s, prods = [], [], [], [], []
        for k in range(3):
            f = plane1(f"F{k}")
            nc.vector.scalar_tensor_tensor(
                f[:], sigs[k][:], coef(C_HALF_KA), mp[:], alu.mult, alu.subtract)
            Fs.append(f)
        for k in range(3):
            z = plane1(f"z{k}")
            nc.vector.tensor_tensor(z[:], pvars[k][:], Fs[k][:], alu.mult)
            zs.append(z)
            wv = plane1(f"w{k}")
            nc.vector.tensor_tensor(wv[:], np_[:], qvars[k][:], alu.mult)
            ws.append(wv)
        for i in range(3):
            x = plane1(f"x1{i}")
            nc.vector.tensor_tensor(x[:], mp[:], avars[i][:], alu.mult)
            x1s.append(x)
        prod_ins = ((p2, p3), (p1, p3), (p1, p2))
        for k in range(3):
            pr = plane1(f"prod{k}")
            nc.gpsimd.tensor_tensor(pr[:], prod_ins[k][0][:], prod_ins[k][1][:],
                                    alu.mult)
            prods.append(pr)

        # ---- tau accumulation in PSUM + evacuation ---------------------
        ot = out_pool.tile([128, 6, G, W], f32, tag="ot", name="ot")
        own = {0: (0, 1), 1: (0, 2), 2: (1, 2)}
        opp = {0: 2, 1: 1, 2: 0}
        for i in range(3):
            tb = ps_tau.tile([128, G, W], f32, tag="tau", name="tau_t")
            chain = [(W_N2I, x1s[i])]
            for j in range(3):
                chain.append((W_23I if j == i else W_N13I, Asq[j]))
            for k in own[i]:
                chain.append((W_13I, Psq[k]))
            chain.append((W_N23I, Psq[opp[i]]))
            for j, (wslot, t) in enumerate(chain):
                nc.tensor.matmul(tb[:], wmat(wslot), t[:], start=(j == 0),
                                 stop=(j == len(chain) - 1))
            nc.scalar.copy(ot[:, i, :, :], tb[:])
        for k in range(3):
            tb = ps_tau.tile([128, G, W], f32, tag="tau", name="tau_t")
            chain = [(W_I, zs[k]), (W_NI, ws[k]), (W_KA4, prods[k])]
            for j, (wslot, t) in enumerate(chain):
                nc.tensor.matmul(tb[:], wmat(wslot), t[:], start=(j == 0),
                                 stop=(j == len(chain) - 1))
            nc.scalar.copy(ot[:, 3 + k, :, :], tb[:])

        for k in range(6):
            nc.sync.dma_start(
                tau_ap[k, d0:d0 + G, :, :].rearrange("d h w -> h d w"),
                ot[:, k])


def _get_compiled():
    global _compiled
    if _compiled is not None:
        return _compiled
    import concourse.bacc as bacc
    import concourse.tile as tile
    import concourse.mybir as mybir

    f32 = mybir.dt.float32
    nc = bacc.Bacc("TRN2", target_bir_lowering=False, debug=False,
                   num_devices=NCORES)
    v_ap = nc.dram_tensor("v", [C, SLAB_D, H, WP], f32,
                          kind="ExternalInput").ap()
    wm_ap = nc.dram_tensor("wm", [128, 10 * 128], f32,
                           kind="ExternalInput").ap()
    cf_ap = nc.dram_tensor("cf", [128, 8], f32, kind="ExternalInput").ap()
    tau_ap = nc.dram_tensor("tau", [6, DPC, H, W], f32,
                            kind="ExternalOutput").ap()
    from contextlib import ExitStack
    with tile.TileContext(nc) as tc:
        with ExitStack() as ctx:
            _trace_body(ctx, tc, v_ap, wm_ap, cf_ap, tau_ap)
    nc.compile()
    _compiled = nc
    return nc


last_exec_time_ns = None


def _ensure_ntff_hook():
    """The agent image's antenv lacks axon_hooks; shim it and install the
    ctypes NTFF hook from trn_boot so trace=True yields exec_time_ns."""
    import sys
    import types
    try:
        from antenv.axon_hooks import get_axon_ntff_profile_hook
        if get_axon_ntff_profile_hook() is not None:
            return
    except ImportError:
        mod = types.ModuleType("antenv.axon_hooks")
        _state = {"hook": None}
        mod.set_axon_ntff_profile_hook = lambda h: _state.update(hook=h)
        mod.get_axon_ntff_profile_hook = lambda: _state["hook"]
        import antenv
        antenv.axon_hooks = mod
        sys.modules["antenv.axon_hooks"] = mod
    from antenv.axon_hooks import set_axon_ntff_profile_hook
    from trn_agent_boot.trn_boot import _ntff_profile_via_ctypes
    set_axon_ntff_profile_hook(_ntff_profile_via_ctypes("/opt/axon/libaxon_pjrt.so"))
    # upload_artifacts does a network fish copy; neuter it for local runs
    from concourse import bass_utils
    bass_utils.upload_artifacts = lambda tmpdir: tmpdir


def kernel(velocity, cs_raw, cr_raw, ca_raw, trace=False):
    global last_exec_time_ns
    from concourse.bass_utils import run_bass_kernel_spmd

    velocity = np.asarray(velocity, dtype=np.float32)
    s = D / 2.0
    cs = _softplus(cs_raw) * 0.2
    cr = _softplus(cr_raw) * 0.1
    ca = _softplus(ca_raw) * 0.05
    P_d = 2.0 * cs * cs * s * s
    P_r = 2.0 * cr * cr * s * s
    K_a = ca * s * s
    epsp = EPS / (s * s)

    wm = _build_wm(K_a)
    cf = _build_cf(P_d, P_r, K_a, epsp)

    vp = np.pad(velocity, ((0, 0), (0, 0), (1, 1), (0, 0), (1, 1)),
                mode="edge")
    in_maps = []
    for i in range(NCORES):
        b = i // (NCORES // B)
        d0 = (i % (NCORES // B)) * DPC
        slab = np.ascontiguousarray(vp[b, :, d0:d0 + SLAB_D, :, :])
        in_maps.append({"v": slab, "wm": wm, "cf": cf})

    if trace:
        _ensure_ntff_hook()
    nc = _get_compiled()
    res = run_bass_kernel_spmd(nc, in_maps, core_ids=list(range(NCORES)),
                               trace=trace)
    if res.exec_time_ns is not None or trace:
        last_exec_time_ns = res.exec_time_ns

    out = np.empty((B, 6, D, H, W), np.float32)
    for i in range(NCORES):
        b = i // (NCORES // B)
        d0 = (i % (NCORES // B)) * DPC
        out[b, :, d0:d0 + DPC] = res.results[i]["tau"]
    return out
